# revision 1
# baseline (speedup 1.0000x reference)
"""Trainium2 Bass kernel for nn_CrossmotionModule (gnn_message_passing).

Reference computation (B=4, M=256, T=64, Dm=512, E=768):
    rel[b,m,t,n,k] = (c[b,m,t,k] - c[b,n,t,k]) * vis[b,m,t] * vis[b,n,t]
    fea[b,t,m,(n,k)] = rel                  # (B,T,M,512)
    h   = fea @ W1 + b1                     # (B,T,M,512)
    out = [h, pos] @ W2 + b2                # (B,T,M,768)

Algebraic collapse: with p = vis (B,T,M), u0 = p*c0, u1 = p*c1, the output is
a rank-3 outer product per (b,t) plus a constant:
    out[bt,m,e] = u0[m]*G0[e] + u1[m]*G1[e] - p[m]*G2[e] + const[m,e]
where, with the host-folded fused weight V2 = W1 @ W2[:512] (512, 768):
    G0[e] = sum_n p[n] V2[2n, e]
    G1[e] = sum_n p[n] V2[2n+1, e]
    G2[e] = sum_nk (p*c)[nk] V2[nk, e]
    const = b1 @ W2[:512] + pos @ W2[512:] + b2

All matmuls run single-pass bf16 with exact split compensation
(x = xh + xl, both bf16; dropped xl*yl term is ~2^-16 relative), so the
result matches fp32 to ~1e-5 while avoiding the 2-pass fp32 PE mode.

Sharding: data-parallel over bt = (b,t) flattened; 256 rows / 8 cores = 32
rows per core. Weights replicated. No cross-device communication.
"""

import ml_dtypes
import numpy as np

B, M, T = 4, 256, 64
D_MOT, D_ABS, D_OUT = 512, 512, 768
N_CORES = 8
BT = B * T            # 256
R = BT // N_CORES     # 32 bt rows per core
E = D_OUT
RT = 2                # bt rows per output tile/DMA

BF16 = ml_dtypes.bfloat16

_CACHED_NC = None


def _split_bf16(x):
    xh = x.astype(BF16)
    xl = (x - xh.astype(np.float32)).astype(BF16)
    return xh, xl


def _build_nc():
    """Build the SPMD Bass program (identical for all 8 cores)."""
    import concourse.bacc as bacc
    import concourse.bass as bass
    import concourse.mybir as mybir
    import concourse.tile as tile

    f32 = mybir.dt.float32
    bf16 = mybir.dt.bfloat16
    PSUM = bass.MemorySpace.PSUM

    nc = bacc.Bacc("TRN2", target_bir_lowering=False, debug=False)

    # Per-core inputs (host-prepared layouts; see _prep_inputs).
    la_d = nc.dram_tensor("la", [128, 4 * 96], bf16, kind="ExternalInput")
    lb_d = nc.dram_tensor("lb", [128, 4 * 96], bf16, kind="ExternalInput")
    vh_d = nc.dram_tensor("vh", [128, 4 * E], bf16, kind="ExternalInput")
    vl_d = nc.dram_tensor("vl", [128, 4 * E], bf16, kind="ExternalInput")
    ut9_d = nc.dram_tensor("ut9", [9, R * 256], bf16, kind="ExternalInput")
    cst_d = nc.dram_tensor("cst", [128, 1536], f32, kind="ExternalInput")
    out_d = nc.dram_tensor("out", [R, M, E], f32, kind="ExternalOutput")
    # DRAM bounce buffer for the G partition reshuffle, laid out so the
    # read back into (9, R*E) SBUF partitions is one plain fat DMA.
    gscr_d = nc.dram_tensor("gscr", [9, R * E], bf16)

    with tile.TileContext(nc) as tc:
        with tc.tile_pool(name="persist", bufs=1) as pers:
            ut9_sb = pers.tile([9, R * 256], bf16)
            g9_sb = pers.tile([9, R * E], bf16)
            cst_sb = pers.tile([128, 1536], f32)

            # ---- prologue: G[(j,r), e] via the fused weight V2 ----
            with (
                tc.tile_pool(name="pro", bufs=1) as pro,
                tc.tile_pool(name="prop", bufs=1, space=PSUM) as prop,
            ):
                la_sb = pro.tile([128, 4 * 96], bf16)
                lb_sb = pro.tile([128, 4 * 96], bf16)
                vh_sb = pro.tile([128, 4 * E], bf16)
                vl_sb = pro.tile([128, 4 * E], bf16)
                nc.sync.dma_start(vh_sb[:], vh_d[:])
                nc.sync.dma_start(la_sb[:], la_d[:])
                nc.sync.dma_start(lb_sb[:], lb_d[:])
                nc.sync.dma_start(vl_sb[:], vl_d[:])
                nc.sync.dma_start(ut9_sb[:], ut9_d[:])
                nc.sync.dma_start(cst_sb[:], cst_d[:])

                # G = Gh + Gl packed side by side: [Gh | Gl] per row.
                ghl_sb = pro.tile([3 * R, 2 * E], bf16)

                # 12 accumulation steps x 2 PSUM-bank segments:
                #   kk 0-3: lhsT = L chunks,  rhs = V2h chunks
                #   kk 4-7: lhsT = L chunks,  rhs = V2l chunks
                #   kk 8-11: lhsT = LB chunks ([0|0|Ql]), rhs = V2h chunks
                g_ps = prop.tile([3 * R, E], f32)
                for kk in range(12):
                    lsrc = lb_sb if kk >= 8 else la_sb
                    vsrc = vl_sb if 4 <= kk < 8 else vh_sb
                    kc = kk % 4
                    for lo, hi in ((0, 512), (512, 768)):
                        nc.tensor.matmul(
                            g_ps[:, lo:hi],
                            lsrc[:, kc * 96 : (kc + 1) * 96],
                            vsrc[:, kc * E + lo : kc * E + hi],
                            start=(kk == 0),
                            stop=(kk == 11),
                        )
                # Split G into exact bf16 halves: G = Gh + Gl (+ ~2^-16).
                nc.vector.tensor_copy(ghl_sb[:, 0:E], g_ps[:])
                nc.vector.tensor_sub(ghl_sb[:, E : 2 * E], g_ps[:], ghl_sb[:, 0:E])
                # Reshuffle rows (j*R+r, [h|l] e) -> [Gh;Gl;Gh] x (r, e).
                # The scatter happens on the DRAM WRITE side (src keeps 96
                # partitions -> full SDMA parallelism); the read back is one
                # plain (9, R*E) DMA with 48KB-per-partition descriptors.
                nc.sync.dma_start(
                    gscr_d[0:3].rearrange("j (r e) -> (j r) e", r=R),
                    ghl_sb[:, 0:E],
                )
                nc.sync.dma_start(
                    gscr_d[3:6].rearrange("j (r e) -> (j r) e", r=R),
                    ghl_sb[:, E : 2 * E],
                )
                nc.sync.dma_start(
                    gscr_d[6:9].rearrange("j (r e) -> (j r) e", r=R),
                    ghl_sb[:, 0:E],
                )
                # Read back in 4 r-contiguous chunks so the first main-loop
                # matmuls only wait on chunk 0 (region-level tile deps).
                CK = R // 4 * E
                for ck in range(4):
                    nc.sync.dma_start(
                        g9_sb[:, ck * CK : (ck + 1) * CK],
                        gscr_d[:, ck * CK : (ck + 1) * CK],
                    )

            # ---- main loop: out[r, m, e] = U9_r^T G9_r + const ----
            # Output tiles: RT rows per DMA, except the last 4 rows which go
            # one per DMA so the tail (last TT -> last byte) is short.
            groups = [
                list(range(g, g + RT)) for g in range(0, R - 4, RT)
            ] + [[r] for r in range(R - 4, R)]
            with (
                tc.tile_pool(name="mp", bufs=2, space=PSUM) as mp,
                tc.tile_pool(name="op", bufs=4) as op,
            ):
                for gi, grp in enumerate(groups):
                    nq = len(grp)
                    halved = gi == 0 or gi >= len(groups) - 2
                    out_sb = op.tile([128, nq * 1536], f32, tag="out_sb")
                    for q, r in enumerate(grp):
                        ps = mp.tile([128, 1536], f32)
                        u0 = ut9_sb[:, r * 256 : r * 256 + 128]
                        u1 = ut9_sb[:, r * 256 + 128 : r * 256 + 256]
                        g0 = r * E
                        nc.tensor.matmul(ps[:, 0:512], u0, g9_sb[:, g0 : g0 + 512])
                        nc.tensor.matmul(ps[:, 512:768], u0, g9_sb[:, g0 + 512 : g0 + 768])
                        nc.tensor.matmul(ps[:, 768:1024], u1, g9_sb[:, g0 : g0 + 256])
                        nc.tensor.matmul(ps[:, 1024:1536], u1, g9_sb[:, g0 + 256 : g0 + 768])
                        if halved:
                            # Half-row epilogue granularity: the final DMAs
                            # issue earlier and the last exposed transfer is
                            # half the size.
                            for h in range(2):
                                sl = slice(q * 1536 + h * 768, q * 1536 + (h + 1) * 768)
                                nc.vector.tensor_add(
                                    out_sb[:, sl], ps[:, h * 768 : (h + 1) * 768],
                                    cst_sb[:, h * 768 : (h + 1) * 768],
                                )
                                nc.sync.dma_start(
                                    out_d[r].rearrange("(p w) e -> w p e", w=2)[h],
                                    out_sb[:, sl],
                                )
                        else:
                            nc.vector.tensor_add(
                                out_sb[:, q * 1536 : (q + 1) * 1536], ps[:], cst_sb[:]
                            )
                    if not halved:
                        nc.sync.dma_start(
                            out_d[grp[0] : grp[0] + nq].rearrange(
                                "q (p w) e -> p q w e", w=2
                            ),
                            out_sb[:, 0 : nq * 1536].rearrange(
                                "p (q w e) -> p q w e", q=nq, w=2
                            ),
                        )
    nc.compile()
    return nc


def _prep_inputs(coords, mask, pos, w1, b1, w2, b2):
    """Host-side input sharding + weight-only constant folding."""
    nan0 = np.isnan(coords[..., 0])
    c = np.nan_to_num(coords)
    vis = np.where(nan0, np.float32(0.0), mask).astype(np.float32)

    p_all = np.ascontiguousarray(vis.transpose(0, 2, 1)).reshape(BT, M)
    c_bt = np.ascontiguousarray(c.transpose(0, 2, 1, 3)).reshape(BT, M, 2)
    q_all = (p_all[:, :, None] * c_bt).reshape(BT, 2 * M).astype(np.float32)

    W2t = w2[:D_MOT]
    W2b = w2[D_MOT:]
    const = (b1 @ W2t + b2)[None, :] + pos @ W2b          # (M, 768)
    cst_dev = np.ascontiguousarray(
        const.astype(np.float32).reshape(128, 2, D_OUT)
    ).reshape(128, 1536)

    # Fused weight V2 = W1 @ W2t, split into exact bf16 halves, each
    # chunked with 128 contraction rows per chunk.
    v2 = (w1 @ W2t).astype(np.float32)                    # (512, 768)
    v2h, v2l = _split_bf16(v2)
    vh_dev = np.ascontiguousarray(
        v2h.reshape(4, 128, D_OUT).transpose(1, 0, 2)
    ).reshape(128, 4 * D_OUT)
    vl_dev = np.ascontiguousarray(
        v2l.reshape(4, 128, D_OUT).transpose(1, 0, 2)
    ).reshape(128, 4 * D_OUT)

    # U9 rows pair with G9 rows [Gh; Gl; Gh]: [Uh; Uh; Ul].
    u0 = q_all[:, 0::2]
    u1 = q_all[:, 1::2]
    U3 = np.stack([u0, u1, -p_all], axis=0)               # (3, BT, M)
    U3 = U3.reshape(3, BT, 128, 2).transpose(0, 1, 3, 2)  # m = 2p+w
    uh, ul = _split_bf16(U3)
    U9 = np.concatenate([uh, uh, ul], axis=0)             # (9, BT, 2, 128)

    qh_all, ql_all = _split_bf16(q_all)

    in_maps = []
    for i in range(N_CORES):
        rows = slice(i * R, (i + 1) * R)
        ut9_i = np.ascontiguousarray(U9[:, rows]).reshape(9, R * 256)

        # L = [P0 | P1 | Qh] (512, 96): P0[2n]=P^T[n], P0[2n+1]=0; P1 odd rows.
        pc_t = p_all[rows].T                              # (256, R)
        la = np.zeros((512, 96), np.float32)
        la[0::2, 0:32] = pc_t
        la[1::2, 32:64] = pc_t
        la = la.astype(BF16)
        la[:, 64:96] = qh_all[rows].T
        lb = np.zeros((512, 96), BF16)
        lb[:, 64:96] = ql_all[rows].T
        la_i = np.ascontiguousarray(
            la.reshape(4, 128, 96).transpose(1, 0, 2)
        ).reshape(128, 384)
        lb_i = np.ascontiguousarray(
            lb.reshape(4, 128, 96).transpose(1, 0, 2)
        ).reshape(128, 384)
        in_maps.append(
            {
                "la": la_i,
                "lb": lb_i,
                "vh": vh_dev,
                "vl": vl_dev,
                "ut9": ut9_i,
                "cst": cst_dev,
            }
        )
    return in_maps


def _run(inputs, trace=False, trace_kwargs=None):
    from concourse.bass_utils import run_bass_kernel_spmd

    global _CACHED_NC
    if _CACHED_NC is None:
        _CACHED_NC = _build_nc()
    nc = _CACHED_NC

    coords = np.asarray(inputs["point_trajs_gt_coord"], dtype=np.float32)
    mask = np.asarray(inputs["point_trajs_visibility_mask"], dtype=np.float32)
    pos = np.asarray(inputs["pos_embed"], dtype=np.float32)
    w1 = np.asarray(inputs["fc1_w"], dtype=np.float32)
    b1 = np.asarray(inputs["fc1_b"], dtype=np.float32)
    w2 = np.asarray(inputs["fc_out_w"], dtype=np.float32)
    b2 = np.asarray(inputs["fc_out_b"], dtype=np.float32)

    in_maps = _prep_inputs(coords, mask, pos, w1, b1, w2, b2)
    res = run_bass_kernel_spmd(
        nc, in_maps, list(range(N_CORES)), trace=trace, **(trace_kwargs or {})
    )
    shards = [res.results[i]["out"] for i in range(N_CORES)]
    full = np.concatenate(shards, axis=0).reshape(B, T, M, D_OUT)
    return full, res


def kernel(**inputs):
    out, _ = _run(inputs, trace=False)
    return out



# revision 2
# speedup vs baseline: 1.1368x; 1.1368x over previous
"""Trainium2 Bass kernel for nn_CrossmotionModule (gnn_message_passing).

Reference computation (B=4, M=256, T=64, Dm=512, E=768):
    rel[b,m,t,n,k] = (c[b,m,t,k] - c[b,n,t,k]) * vis[b,m,t] * vis[b,n,t]
    fea[b,t,m,(n,k)] = rel                  # (B,T,M,512)
    h   = fea @ W1 + b1                     # (B,T,M,512)
    out = [h, pos] @ W2 + b2                # (B,T,M,768)

Algebraic collapse: with p = vis (B,T,M), u0 = p*c0, u1 = p*c1, the output is
a rank-3 outer product per (b,t) plus a constant:
    out[bt,m,e] = u0[m]*G0[e] + u1[m]*G1[e] - p[m]*G2[e] + const[m,e]
where, with the host-folded fused weight V2 = W1 @ W2[:512] (512, 768):
    G0[e] = sum_n p[n] V2[2n, e]
    G1[e] = sum_n p[n] V2[2n+1, e]
    G2[e] = sum_nk (p*c)[nk] V2[nk, e]
    const = cvec + pos @ W2[512:],  cvec = b1 @ W2[:512] + b2

Single-bf16 everywhere (rel_l2 ~ 3e-3 << 2e-2): all matmul inputs bf16, fp32
PSUM accumulate, output written bf16 and widened to fp32 on host. The
m-independent const part (cvec) is folded into the per-row matmul as a 4th
contraction row (ones x cvec). When pos @ W2[512:] is exactly zero (true for
the reference inputs: pos_embed == 0) nothing else is needed; otherwise a
fallback program variant adds the (m,e)-dependent part on the vector engine.

Sharding: data-parallel over bt = (b,t) flattened; 256 rows / 8 cores = 32
rows per core. Weights replicated. No cross-device communication.
"""

import ml_dtypes
import numpy as np

B, M, T = 4, 256, 64
D_MOT, D_ABS, D_OUT = 512, 512, 768
N_CORES = 8
BT = B * T            # 256
R = BT // N_CORES     # 32 bt rows per core
E = D_OUT
RT = 2                # bt rows per output DMA

BF16 = ml_dtypes.bfloat16

_CACHED_NC = {}


def _build_nc(variant):
    """Build the SPMD Bass program (identical for all 8 cores).

    variant: 'fold' — const is rank-1 (ones x cvec), folded into the matmul.
             'add'  — general const; epilogue adds pcst[m, (w e)] on DVE.
    """
    import concourse.bacc as bacc
    import concourse.bass as bass
    import concourse.mybir as mybir
    import concourse.tile as tile

    f32 = mybir.dt.float32
    bf16 = mybir.dt.bfloat16
    PSUM = bass.MemorySpace.PSUM

    nc = bacc.Bacc("TRN2", target_bir_lowering=False, debug=False)

    la_d = nc.dram_tensor("la", [128, 4 * 96], bf16, kind="ExternalInput")
    vw_d = nc.dram_tensor("vw", [128, 4 * E], bf16, kind="ExternalInput")
    ut4_d = nc.dram_tensor("ut4", [4, R * 256], bf16, kind="ExternalInput")
    cvd_d = nc.dram_tensor("cvd", [1, R * E], bf16, kind="ExternalInput")
    if variant == "add":
        pcst_d = nc.dram_tensor("pcst", [128, 1536], f32, kind="ExternalInput")
    out_d = nc.dram_tensor("out", [R, M, E], bf16, kind="ExternalOutput")

    with tile.TileContext(nc) as tc:
        with tc.tile_pool(name="persist", bufs=1) as pers:
            ut4_sb = pers.tile([4, R * 256], bf16)
            g4_sb = pers.tile([4, R * E], bf16)
            if variant == "add":
                pcst_sb = pers.tile([128, 1536], f32)

            # ---- prologue: G[(j,r), e] = L^T V2 ----
            with (
                tc.tile_pool(name="pro", bufs=1) as pro,
                tc.tile_pool(name="prop", bufs=1, space=PSUM) as prop,
            ):
                la_sb = pro.tile([128, 4 * 96], bf16)
                vw_sb = pro.tile([128, 4 * E], bf16)
                gtmp = pro.tile([96, E], bf16)

                # Input DMAs split across the two HWDGE queues (SP + Act) so
                # descriptor generation does not serialize the critical path.
                nc.sync.dma_start(la_sb[:], la_d[:])
                nc.sync.dma_start(vw_sb[:, 0:E], vw_d[:, 0:E])
                nc.sync.dma_start(vw_sb[:, E : 2 * E], vw_d[:, E : 2 * E])
                nc.scalar.dma_start(vw_sb[:, 2 * E : 3 * E], vw_d[:, 2 * E : 3 * E])
                nc.scalar.dma_start(vw_sb[:, 3 * E : 4 * E], vw_d[:, 3 * E : 4 * E])
                nc.scalar.dma_start(ut4_sb[:], ut4_d[:])
                nc.scalar.dma_start(g4_sb[3:4, :], cvd_d[:])
                if variant == "add":
                    nc.scalar.dma_start(pcst_sb[:], pcst_d[:])

                # G accumulation: 4 contraction chunks of 128, fp32 PSUM.
                gps = prop.tile([96, E], f32)
                for kk in range(4):
                    for lo, hi in ((0, 512), (512, E)):
                        nc.tensor.matmul(
                            gps[:, lo:hi],
                            la_sb[:, kk * 96 : (kk + 1) * 96],
                            vw_sb[:, kk * E + lo : kk * E + hi],
                            start=(kk == 0),
                            stop=(kk == 3),
                        )
                # fp32 -> bf16 cast, split across DVE and Act.
                nc.vector.tensor_copy(gtmp[:, 0:384], gps[:, 0:384])
                nc.scalar.copy(gtmp[:, 384:E], gps[:, 384:E])

                # Reshuffle rows (j*R + r, e) -> (j, r*E + e) with direct
                # SBUF->SBUF DMAs (no DRAM bounce). Two r-chunks per j so the
                # first main-loop rows only wait on the first chunk.
                CK = (R // 2) * E
                for j in range(3):
                    eng = (nc.sync, nc.scalar, nc.sync)[j]
                    for ck in range(2):
                        eng.dma_start(
                            g4_sb[j : j + 1, ck * CK : (ck + 1) * CK],
                            gtmp[j * R + ck * 16 : j * R + (ck + 1) * 16, :],
                        )

            # ---- main loop: out[r, m, e] = U4_r^T G4_r (+ pcst) ----
            with (
                tc.tile_pool(name="mp", bufs=2, space=PSUM) as mp,
                tc.tile_pool(name="op", bufs=4) as op,
            ):
                for g0 in range(0, R, RT):
                    out_sb = op.tile([128, RT * 1536], bf16, tag="out_sb")
                    for q in range(RT):
                        r = g0 + q
                        ps = mp.tile([128, 1536], f32)
                        uw0 = ut4_sb[:, r * 256 : r * 256 + 128]
                        uw1 = ut4_sb[:, r * 256 + 128 : r * 256 + 256]
                        gb = r * E
                        nc.tensor.matmul(ps[:, 0:512], uw0, g4_sb[:, gb : gb + 512])
                        nc.tensor.matmul(ps[:, 512:768], uw0, g4_sb[:, gb + 512 : gb + 768])
                        nc.tensor.matmul(ps[:, 768:1024], uw1, g4_sb[:, gb : gb + 256])
                        nc.tensor.matmul(ps[:, 1024:1536], uw1, g4_sb[:, gb + 256 : gb + 768])
                        o = q * 1536
                        if variant == "add":
                            nc.vector.tensor_add(
                                out_sb[:, o : o + 1536], ps[:], pcst_sb[:]
                            )
                        else:
                            nc.vector.tensor_copy(out_sb[:, o : o + 768], ps[:, 0:768])
                            nc.scalar.copy(out_sb[:, o + 768 : o + 1536], ps[:, 768:1536])
                    nc.sync.dma_start(
                        out_d[g0 : g0 + RT].rearrange("q (p w) e -> p q w e", w=2),
                        out_sb[:].rearrange("p (q w e) -> p q w e", q=RT, w=2),
                    )
    nc.compile()
    return nc


def _prep_inputs(coords, mask, pos, w1, b1, w2, b2):
    """Host-side input sharding + weight-only constant folding."""
    nan0 = np.isnan(coords[..., 0])
    c = np.nan_to_num(coords)
    vis = np.where(nan0, np.float32(0.0), mask).astype(np.float32)

    p_all = np.ascontiguousarray(vis.transpose(0, 2, 1)).reshape(BT, M)
    c_bt = np.ascontiguousarray(c.transpose(0, 2, 1, 3)).reshape(BT, M, 2)
    q_all = (p_all[:, :, None] * c_bt).reshape(BT, 2 * M).astype(np.float32)

    W2t = w2[:D_MOT]
    W2b = w2[D_MOT:]
    cvec = (b1 @ W2t + b2).astype(np.float32)             # (768,)
    pcst = (pos @ W2b).astype(np.float32)                 # (M, 768)
    variant = "fold" if not np.any(pcst) else "add"

    # Fused weight V2 = W1 @ W2t in bf16, 128 contraction rows per chunk.
    v2 = (w1 @ W2t).astype(np.float32)                    # (512, 768)
    vw_dev = np.ascontiguousarray(
        v2.astype(BF16).reshape(4, 128, D_OUT).transpose(1, 0, 2)
    ).reshape(128, 4 * D_OUT)

    # U rows: u0, u1, -p, ones; layout (4, BT, w, 128) with m = 2p + w.
    u0 = q_all[:, 0::2]
    u1 = q_all[:, 1::2]
    ones = np.ones_like(p_all)
    U4 = np.stack([u0, u1, -p_all, ones], axis=0)         # (4, BT, M)
    U4 = U4.reshape(4, BT, 128, 2).transpose(0, 1, 3, 2).astype(BF16)

    cvd = np.broadcast_to(cvec.astype(BF16), (R, E)).reshape(1, R * E).copy()
    pcst_dev = None
    if variant == "add":
        pcst_dev = np.ascontiguousarray(
            pcst.reshape(128, 2, D_OUT)
        ).reshape(128, 1536)

    qb = q_all.astype(BF16)

    in_maps = []
    for i in range(N_CORES):
        rows = slice(i * R, (i + 1) * R)
        ut4_i = np.ascontiguousarray(U4[:, rows]).reshape(4, R * 256)

        # L = [P0 | P1 | Q] (512, 96): P0[2n]=p^T[n], P1[2n+1]=p^T[n], Q = q^T.
        pc_t = p_all[rows].T                              # (256, R)
        la = np.zeros((512, 96), np.float32)
        la[0::2, 0:32] = pc_t
        la[1::2, 32:64] = pc_t
        la = la.astype(BF16)
        la[:, 64:96] = qb[rows].T
        la_i = np.ascontiguousarray(
            la.reshape(4, 128, 96).transpose(1, 0, 2)
        ).reshape(128, 384)
        m = {
            "la": la_i,
            "vw": vw_dev,
            "ut4": ut4_i,
            "cvd": cvd,
        }
        if variant == "add":
            m["pcst"] = pcst_dev
        in_maps.append(m)
    return in_maps, variant


def _run(inputs, trace=False, trace_kwargs=None):
    from concourse.bass_utils import run_bass_kernel_spmd

    coords = np.asarray(inputs["point_trajs_gt_coord"], dtype=np.float32)
    mask = np.asarray(inputs["point_trajs_visibility_mask"], dtype=np.float32)
    pos = np.asarray(inputs["pos_embed"], dtype=np.float32)
    w1 = np.asarray(inputs["fc1_w"], dtype=np.float32)
    b1 = np.asarray(inputs["fc1_b"], dtype=np.float32)
    w2 = np.asarray(inputs["fc_out_w"], dtype=np.float32)
    b2 = np.asarray(inputs["fc_out_b"], dtype=np.float32)

    in_maps, variant = _prep_inputs(coords, mask, pos, w1, b1, w2, b2)
    if variant not in _CACHED_NC:
        _CACHED_NC[variant] = _build_nc(variant)
    nc = _CACHED_NC[variant]

    res = run_bass_kernel_spmd(
        nc, in_maps, list(range(N_CORES)), trace=trace, **(trace_kwargs or {})
    )
    shards = [np.asarray(res.results[i]["out"]) for i in range(N_CORES)]
    full = np.concatenate(shards, axis=0).astype(np.float32).reshape(B, T, M, D_OUT)
    return full, res


def kernel(**inputs):
    out, _ = _run(inputs, trace=False)
    return out


# revision 5
# speedup vs baseline: 1.3520x; 1.1893x over previous
"""Trainium2 Bass kernel for nn_CrossmotionModule (gnn_message_passing).

Reference computation (B=4, M=256, T=64, Dm=512, E=768):
    rel[b,m,t,n,k] = (c[b,m,t,k] - c[b,n,t,k]) * vis[b,m,t] * vis[b,n,t]
    fea[b,t,m,(n,k)] = rel                  # (B,T,M,512)
    h   = fea @ W1 + b1                     # (B,T,M,512)
    out = [h, pos] @ W2 + b2                # (B,T,M,768)

Algebraic collapse: with p = vis (B,T,M), u0 = p*c0, u1 = p*c1, the output is
a rank-3 outer product per (b,t) plus a constant:
    out[bt,m,e] = u0[m]*G0[e] + u1[m]*G1[e] - p[m]*G2[e] + const[m,e]
where, with the host-folded fused weight V2 = W1 @ W2[:512] (512, 768):
    G_j = [P0 | P1 | Q]_j^T V2  (bf16 inputs, fp32 PSUM accumulate)
    const = cvec + pos @ W2[512:],  cvec = b1 @ W2[:512] + b2

Main loop runs in split-fp8 (e4m3) with the PE DoubleRow perf mode (2 fp8
contraction rows per partition per cycle, 0.5 cycles per output column):
u = uh8 + ul8 and G = gh8 + gl8, keeping all four cross products, plus the
rank-1 const fold (ones x cvec split into 2 fp8 rows) — 14 contraction rows
in 7 partitions x 2 sub-rows. rel_l2 ~ 4.7e-3 (gate 2e-2). Output is written
bf16 and widened to fp32 on host.

When pos @ W2[512:] is nonzero a fallback bf16 variant adds the (m,e) const
on the vector engine instead (correct for any input, slower).

Sharding: data-parallel over bt = (b,t); 256 rows / 8 cores = 32 per core.
Weights replicated; no cross-device communication.
"""

import ml_dtypes
import numpy as np

B, M, T = 4, 256, 64
D_MOT, D_ABS, D_OUT = 512, 512, 768
N_CORES = 8
BT = B * T            # 256
R = BT // N_CORES     # 32 bt rows per core
E = D_OUT
RT = 2

BF16 = ml_dtypes.bfloat16
F8 = ml_dtypes.float8_e4m3fn

_CACHED_NC = {}


def _build_nc(variant):
    """variant: 'fold' — fp8 DoubleRow fast path (const is rank-1).
                'add'  — bf16 general path; epilogue adds pcst on DVE."""
    import concourse.bacc as bacc
    import concourse.bass as bass
    import concourse.mybir as mybir
    import concourse.tile as tile

    f32 = mybir.dt.float32
    bf16 = mybir.dt.bfloat16
    fp8 = mybir.dt.float8e4
    DR = mybir.MatmulPerfMode.DoubleRow
    PSUM = bass.MemorySpace.PSUM

    nc = bacc.Bacc("TRN2", target_bir_lowering=False, debug=False)

    la_d = nc.dram_tensor("la", [128, 4 * 96], bf16, kind="ExternalInput")
    vw_d = nc.dram_tensor("vw", [128, 4 * E], bf16, kind="ExternalInput")
    if variant == "fold":
        ut_d = nc.dram_tensor("ut8", [7, R * 512], fp8, kind="ExternalInput")
        cv_d = nc.dram_tensor("cv8", [1, R * 1536], fp8, kind="ExternalInput")
    else:
        ut_d = nc.dram_tensor("ut4", [4, R * 256], bf16, kind="ExternalInput")
        cv_d = nc.dram_tensor("cvd", [1, R * E], bf16, kind="ExternalInput")
        pcst_d = nc.dram_tensor("pcst", [128, 1536], f32, kind="ExternalInput")
    out_d = nc.dram_tensor("out", [R, M, E], bf16, kind="ExternalOutput")

    with tile.TileContext(nc) as tc:
        with tc.tile_pool(name="persist", bufs=1) as pers:
            if variant == "fold":
                ut_sb = pers.tile([7, R * 512], fp8)
                g8_sb = pers.tile([7, R * 1536], fp8)
            else:
                ut_sb = pers.tile([4, R * 256], bf16)
                g4_sb = pers.tile([4, R * E], bf16)
                pcst_sb = pers.tile([128, 1536], f32)

            # ---- prologue: G[(j,r), e] = L^T V2 (bf16, fp32 accumulate) ----
            with (
                tc.tile_pool(name="pro", bufs=1) as pro,
                tc.tile_pool(name="prop", bufs=1, space=PSUM) as prop,
            ):
                la_sb = pro.tile([128, 4 * 96], bf16)
                vw_sb = pro.tile([128, 4 * E], bf16)

                nc.sync.dma_start(vw_sb[:, 0:E], vw_d[:, 0:E])
                nc.sync.dma_start(la_sb[:], la_d[:])
                nc.sync.dma_start(vw_sb[:, E : 2 * E], vw_d[:, E : 2 * E])
                nc.scalar.dma_start(vw_sb[:, 2 * E : 3 * E], vw_d[:, 2 * E : 3 * E])
                nc.scalar.dma_start(vw_sb[:, 3 * E : 4 * E], vw_d[:, 3 * E : 4 * E])
                nc.scalar.dma_start(ut_sb[:], ut_d[:])
                if variant == "fold":
                    nc.scalar.dma_start(g8_sb[6:7, :], cv_d[:])
                else:
                    nc.scalar.dma_start(g4_sb[3:4, :], cv_d[:])
                    nc.scalar.dma_start(pcst_sb[:], pcst_d[:])

                gps = prop.tile([96, E], f32)
                for kk in range(4):
                    for lo, hi in ((0, 512), (512, E)):
                        nc.tensor.matmul(
                            gps[:, lo:hi],
                            la_sb[:, kk * 96 : (kk + 1) * 96],
                            vw_sb[:, kk * E + lo : kk * E + hi],
                            start=(kk == 0),
                            stop=(kk == 3),
                        )

                if variant == "fold":
                    # Split G into fp8 high/low: gh = fp8(G), gl = fp8(G - gh).
                    gh_sb = pro.tile([96, E], fp8)
                    gl_sb = pro.tile([96, E], fp8)
                    nc.scalar.copy(gh_sb[:], gps[:])
                    nc.vector.tensor_sub(gl_sb[:], gps[:], gh_sb[:])
                    # Reshuffle (j*R + r, e) -> [kp, r*1536 + sub*768 + e]:
                    # kp 0-2 pair (uh, gh|gl), kp 3-5 pair (ul, gh|gl).
                    for kp0, sub, src in (
                        (0, 0, gh_sb[:]),
                        (3, 0, gh_sb[:]),
                        (0, 1, gl_sb[:]),
                        (3, 1, gl_sb[:]),
                    ):
                        eng = nc.sync if sub == 0 else nc.scalar
                        eng.dma_start(
                            g8_sb[kp0 : kp0 + 3, :].rearrange(
                                "k (r s e) -> k r s e", r=R, s=2
                            )[:, :, sub, :],
                            src,
                        )
                else:
                    gtmp = pro.tile([96, E], bf16)
                    nc.vector.tensor_copy(gtmp[:, 0:384], gps[:, 0:384])
                    nc.scalar.copy(gtmp[:, 384:E], gps[:, 384:E])
                    CK = (R // 2) * E
                    for j in range(3):
                        eng = (nc.sync, nc.scalar, nc.sync)[j]
                        for ck in range(2):
                            eng.dma_start(
                                g4_sb[j : j + 1, ck * CK : (ck + 1) * CK],
                                gtmp[j * R + ck * 16 : j * R + (ck + 1) * 16, :],
                            )

            # ---- main loop ----
            groups = [list(range(g, g + RT)) for g in range(0, R - 2, RT)] + [
                [R - 2],
                [R - 1],
            ]
            with (
                tc.tile_pool(name="mp", bufs=2, space=PSUM) as mp,
                tc.tile_pool(name="op", bufs=4) as op,
            ):
                for grp in groups:
                    nq = len(grp)
                    out_sb = op.tile([128, nq * 1536], bf16, tag="out_sb")
                    for q, r in enumerate(grp):
                        o = q * 1536
                        if variant == "fold":
                            gv = g8_sb[:, r * 1536 : (r + 1) * 1536].rearrange(
                                "k (s e) -> k s e", s=2
                            )
                            uv0 = ut_sb[:, r * 512 : r * 512 + 256].rearrange(
                                "k (s m) -> k s m", s=2
                            )
                            uv1 = ut_sb[:, r * 512 + 256 : r * 512 + 512].rearrange(
                                "k (s m) -> k s m", s=2
                            )
                            psa = mp.tile([128, 512], f32)
                            psb = mp.tile([128, 512], f32)
                            psc = mp.tile([128, 512], f32)
                            nc.tensor.matmul(psa[:], uv0, gv[:, :, 0:512], perf_mode=DR)
                            nc.tensor.matmul(
                                psb[:, 0:256], uv0, gv[:, :, 512:768], perf_mode=DR
                            )
                            nc.tensor.matmul(
                                psb[:, 256:512], uv1, gv[:, :, 0:256], perf_mode=DR
                            )
                            nc.tensor.matmul(psc[:], uv1, gv[:, :, 256:768], perf_mode=DR)
                            e1, e2 = (nc.vector, nc.scalar) if r % 2 == 0 else (
                                nc.scalar,
                                nc.vector,
                            )
                            for eng, ps, c0 in (
                                (e1, psa, 0),
                                (e2, psb, 512),
                                (e1, psc, 1024),
                            ):
                                if eng is nc.vector:
                                    eng.tensor_copy(out_sb[:, o + c0 : o + c0 + 512], ps[:])
                                else:
                                    eng.copy(out_sb[:, o + c0 : o + c0 + 512], ps[:])
                        else:
                            ps = mp.tile([128, 1536], f32)
                            uw0 = ut_sb[:, r * 256 : r * 256 + 128]
                            uw1 = ut_sb[:, r * 256 + 128 : r * 256 + 256]
                            gb = r * E
                            nc.tensor.matmul(ps[:, 0:512], uw0, g4_sb[:, gb : gb + 512])
                            nc.tensor.matmul(
                                ps[:, 512:768], uw0, g4_sb[:, gb + 512 : gb + 768]
                            )
                            nc.tensor.matmul(
                                ps[:, 768:1024], uw1, g4_sb[:, gb : gb + 256]
                            )
                            nc.tensor.matmul(
                                ps[:, 1024:1536], uw1, g4_sb[:, gb + 256 : gb + 768]
                            )
                            nc.vector.tensor_add(
                                out_sb[:, o : o + 1536], ps[:], pcst_sb[:]
                            )
                    nc.sync.dma_start(
                        out_d[grp[0] : grp[0] + nq].rearrange("q (p w) e -> p q w e", w=2),
                        out_sb[:].rearrange("p (q w e) -> p q w e", q=nq, w=2),
                    )
    nc.compile()
    return nc


def _split8(x):
    xh = x.astype(F8)
    xl = (x - xh.astype(np.float32)).astype(F8)
    return xh, xl


def _prep_inputs(coords, mask, pos, w1, b1, w2, b2):
    nan0 = np.isnan(coords[..., 0])
    c = np.nan_to_num(coords)
    vis = np.where(nan0, np.float32(0.0), mask).astype(np.float32)

    p_all = np.ascontiguousarray(vis.transpose(0, 2, 1)).reshape(BT, M)
    c_bt = np.ascontiguousarray(c.transpose(0, 2, 1, 3)).reshape(BT, M, 2)
    q_all = (p_all[:, :, None] * c_bt).reshape(BT, 2 * M).astype(np.float32)

    W2t = w2[:D_MOT]
    W2b = w2[D_MOT:]
    cvec = (b1 @ W2t + b2).astype(np.float32)
    pcst = (pos @ W2b).astype(np.float32)
    variant = "fold" if not np.any(pcst) else "add"

    v2 = (w1 @ W2t).astype(np.float32)
    vw_dev = np.ascontiguousarray(
        v2.astype(BF16).reshape(4, 128, D_OUT).transpose(1, 0, 2)
    ).reshape(128, 4 * D_OUT)

    u0 = q_all[:, 0::2]
    u1 = q_all[:, 1::2]
    U3 = np.stack([u0, u1, -p_all], axis=0)               # (3, BT, M)
    U3w = U3.reshape(3, BT, 128, 2).transpose(0, 1, 3, 2)  # (3, BT, w, p)

    if variant == "fold":
        uh, ul = _split8(U3w.astype(np.float32))
        # ut8[kp, r, w, sub, p]: kp 0-2 = uh_j, 3-5 = ul_j, 6 = ones.
        ut = np.zeros((7, BT, 2, 2, 128), F8)
        ut[0:3, :, :, 0, :] = uh
        ut[0:3, :, :, 1, :] = uh
        ut[3:6, :, :, 0, :] = ul
        ut[3:6, :, :, 1, :] = ul
        ut[6] = np.float32(1.0)
        ch, cl = _split8(cvec)
        cv = np.zeros((1, R, 2, E), F8)
        cv[0, :, 0, :] = ch
        cv[0, :, 1, :] = cl
        cv = cv.reshape(1, R * 1536)
    else:
        ut = U3w.astype(BF16)
        ones = np.ones((1, BT, 2, 128), BF16)
        ut = np.concatenate([ut, ones], axis=0)           # (4, BT, w, p)
        cv = np.broadcast_to(cvec.astype(BF16), (R, E)).reshape(1, R * E).copy()
        pcst_dev = np.ascontiguousarray(pcst.reshape(128, 2, D_OUT)).reshape(128, 1536)

    qb = q_all.astype(BF16)

    in_maps = []
    for i in range(N_CORES):
        rows = slice(i * R, (i + 1) * R)
        pc_t = p_all[rows].T
        la = np.zeros((512, 96), np.float32)
        la[0::2, 0:32] = pc_t
        la[1::2, 32:64] = pc_t
        la = la.astype(BF16)
        la[:, 64:96] = qb[rows].T
        la_i = np.ascontiguousarray(
            la.reshape(4, 128, 96).transpose(1, 0, 2)
        ).reshape(128, 384)
        m = {"la": la_i, "vw": vw_dev}
        if variant == "fold":
            m["ut8"] = np.ascontiguousarray(ut[:, rows]).reshape(7, R * 512)
            m["cv8"] = cv
        else:
            m["ut4"] = np.ascontiguousarray(ut[:, rows]).reshape(4, R * 256)
            m["cvd"] = cv
            m["pcst"] = pcst_dev
        in_maps.append(m)
    return in_maps, variant


def _run(inputs, trace=False, trace_kwargs=None):
    from concourse.bass_utils import run_bass_kernel_spmd

    coords = np.asarray(inputs["point_trajs_gt_coord"], dtype=np.float32)
    mask = np.asarray(inputs["point_trajs_visibility_mask"], dtype=np.float32)
    pos = np.asarray(inputs["pos_embed"], dtype=np.float32)
    w1 = np.asarray(inputs["fc1_w"], dtype=np.float32)
    b1 = np.asarray(inputs["fc1_b"], dtype=np.float32)
    w2 = np.asarray(inputs["fc_out_w"], dtype=np.float32)
    b2 = np.asarray(inputs["fc_out_b"], dtype=np.float32)

    in_maps, variant = _prep_inputs(coords, mask, pos, w1, b1, w2, b2)
    if variant not in _CACHED_NC:
        _CACHED_NC[variant] = _build_nc(variant)
    nc = _CACHED_NC[variant]

    res = run_bass_kernel_spmd(
        nc, in_maps, list(range(N_CORES)), trace=trace, **(trace_kwargs or {})
    )
    shards = [np.asarray(res.results[i]["out"]) for i in range(N_CORES)]
    full = np.concatenate(shards, axis=0).astype(np.float32).reshape(B, T, M, D_OUT)
    return full, res


def kernel(**inputs):
    out, _ = _run(inputs, trace=False)
    return out


# revision 7
# speedup vs baseline: 1.3789x; 1.0199x over previous
"""Trainium2 Bass kernel for nn_CrossmotionModule (gnn_message_passing).

Reference computation (B=4, M=256, T=64, Dm=512, E=768):
    rel[b,m,t,n,k] = (c[b,m,t,k] - c[b,n,t,k]) * vis[b,m,t] * vis[b,n,t]
    fea[b,t,m,(n,k)] = rel                  # (B,T,M,512)
    h   = fea @ W1 + b1                     # (B,T,M,512)
    out = [h, pos] @ W2 + b2                # (B,T,M,768)

Algebraic collapse: with p = vis (B,T,M), u0 = p*c0, u1 = p*c1, the output is
a rank-3 outer product per (b,t) plus a constant:
    out[bt,m,e] = u0[m]*G0[e] + u1[m]*G1[e] - p[m]*G2[e] + const[m,e]
where, with the host-folded fused weight V2 = W1 @ W2[:512] (512, 768):
    G_j = [P0 | P1 | Q]_j^T V2  (bf16 inputs, fp32 PSUM accumulate)
    const = cvec + pos @ W2[512:],  cvec = b1 @ W2[:512] + b2

Everything runs in single bf16 (fp32 PSUM accumulate); the m-independent
const (cvec) is folded into the per-row matmul as a 4th contraction row
(ones x cvec). Output is written bf16 and widened to fp32 on host
(rel_l2 ~ 3.3e-3, gate 2e-2). The per-row PSUM is drained in three 512-col
bank-aligned chunks by three different engines (DVE / Act / GpSimd) so the
drain never caps the tensor engine. A few warmup matmuls on scratch data
keep the PE p-state up through the input-DMA wait.

When pos @ W2[512:] is nonzero a fallback variant adds the (m,e)-dependent
const on the vector engine instead (correct for any input, slower).

Sharding: data-parallel over bt = (b,t); 256 rows / 8 cores = 32 per core.
Weights replicated; no cross-device communication.
"""

import ml_dtypes
import numpy as np

B, M, T = 4, 256, 64
D_MOT, D_ABS, D_OUT = 512, 512, 768
N_CORES = 8
BT = B * T            # 256
R = BT // N_CORES     # 32 bt rows per core
E = D_OUT
RT = 2

BF16 = ml_dtypes.bfloat16

_CACHED_NC = {}


def _build_nc(variant):
    """variant: 'fold' — const is rank-1, folded into the matmul.
                'add'  — general const; epilogue adds pcst on DVE."""
    import concourse.bacc as bacc
    import concourse.bass as bass
    import concourse.mybir as mybir
    import concourse.tile as tile

    f32 = mybir.dt.float32
    bf16 = mybir.dt.bfloat16
    PSUM = bass.MemorySpace.PSUM

    nc = bacc.Bacc("TRN2", target_bir_lowering=False, debug=False)

    la_d = nc.dram_tensor("la", [128, 4 * 96], bf16, kind="ExternalInput")
    vw_d = nc.dram_tensor("vw", [128, 4 * E], bf16, kind="ExternalInput")
    ut_d = nc.dram_tensor("ut4", [4, R * 256], bf16, kind="ExternalInput")
    cv_d = nc.dram_tensor("cvd", [1, R * E], bf16, kind="ExternalInput")
    if variant == "add":
        pcst_d = nc.dram_tensor("pcst", [128, 1536], f32, kind="ExternalInput")
    out_d = nc.dram_tensor("out", [R, M, E], bf16, kind="ExternalOutput")

    with tile.TileContext(nc) as tc:
        with tc.tile_pool(name="persist", bufs=1) as pers:
            ut_sb = pers.tile([4, R * 256], bf16)
            g4_sb = pers.tile([4, R * E], bf16)
            if variant == "add":
                pcst_sb = pers.tile([128, 1536], f32)

            # ---- prologue: G[(j,r), e] = L^T V2 ----
            with (
                tc.tile_pool(name="pro", bufs=1) as pro,
                tc.tile_pool(name="prop", bufs=1, space=PSUM) as prop,
            ):
                la_sb = pro.tile([128, 4 * 96], bf16)
                vw_sb = pro.tile([128, 4 * E], bf16)
                gtmp = pro.tile([96, E], bf16)
                warm = pro.tile([1, 512], bf16)

                # Input DMAs split across both HWDGE queues; vw chunks first
                # and alternating so the G matmul is fed as early as possible.
                nc.sync.dma_start(vw_sb[:, 0:E], vw_d[:, 0:E])
                nc.scalar.dma_start(vw_sb[:, E : 2 * E], vw_d[:, E : 2 * E])
                nc.sync.dma_start(vw_sb[:, 2 * E : 3 * E], vw_d[:, 2 * E : 3 * E])
                nc.scalar.dma_start(vw_sb[:, 3 * E : 4 * E], vw_d[:, 3 * E : 4 * E])
                nc.sync.dma_start(la_sb[:], la_d[:])
                nc.scalar.dma_start(ut_sb[:], ut_d[:])
                nc.scalar.dma_start(g4_sb[3:4, :], cv_d[:])
                if variant == "add":
                    nc.scalar.dma_start(pcst_sb[:], pcst_d[:])

                # PE warmup: keep the tensor engine busy (and its p-state
                # ramping) across the input-DMA wait. Harmless matmuls on a
                # zeroed scratch tile; results never read.
                wps = prop.tile([1, 512], f32)
                nc.gpsimd.memset(warm[:], 0.0)
                for _ in range(7):
                    nc.tensor.matmul(wps[:], warm[0:1, 0:1], warm[0:1, :])

                gps = prop.tile([96, E], f32)
                for kk in range(4):
                    for lo, hi in ((0, 512), (512, E)):
                        nc.tensor.matmul(
                            gps[:, lo:hi],
                            la_sb[:, kk * 96 : (kk + 1) * 96],
                            vw_sb[:, kk * E + lo : kk * E + hi],
                            start=(kk == 0),
                            stop=(kk == 3),
                        )
                nc.vector.tensor_copy(gtmp[:, 0:384], gps[:, 0:384])
                nc.scalar.copy(gtmp[:, 384:E], gps[:, 384:E])

                # Reshuffle rows (j*R + r, e) -> (j, r*E + e), SBUF->SBUF,
                # r-chunked so early rows unblock first.
                CK = (R // 2) * E
                for j in range(3):
                    eng = (nc.sync, nc.scalar, nc.sync)[j]
                    for ck in range(2):
                        eng.dma_start(
                            g4_sb[j : j + 1, ck * CK : (ck + 1) * CK],
                            gtmp[j * R + ck * 16 : j * R + (ck + 1) * 16, :],
                        )

            # ---- main loop ----
            groups = [list(range(g, g + RT)) for g in range(0, R - 2, RT)] + [
                [R - 2],
                [R - 1],
            ]
            with (
                tc.tile_pool(name="mp", bufs=2, space=PSUM) as mp,
                tc.tile_pool(name="op", bufs=4) as op,
            ):
                for grp in groups:
                    nq = len(grp)
                    out_sb = op.tile([128, nq * 1536], bf16, tag="out_sb")
                    for q, r in enumerate(grp):
                        o = q * 1536
                        uw0 = ut_sb[:, r * 256 : r * 256 + 128]
                        uw1 = ut_sb[:, r * 256 + 128 : r * 256 + 256]
                        gb = r * E
                        if variant == "fold":
                            psa = mp.tile([128, 512], f32)
                            psb = mp.tile([128, 512], f32)
                            psc = mp.tile([128, 512], f32)
                            nc.tensor.matmul(psa[:], uw0, g4_sb[:, gb : gb + 512])
                            nc.tensor.matmul(
                                psb[:, 0:256], uw0, g4_sb[:, gb + 512 : gb + 768]
                            )
                            nc.tensor.matmul(
                                psb[:, 256:512], uw1, g4_sb[:, gb : gb + 256]
                            )
                            nc.tensor.matmul(psc[:], uw1, g4_sb[:, gb + 256 : gb + 768])
                            e1, e2 = (
                                (nc.vector, nc.scalar)
                                if r % 2 == 0
                                else (nc.scalar, nc.vector)
                            )
                            for eng, ps_t, c0 in ((e1, psa, 0), (e2, psb, 512), (e1, psc, 1024)):
                                if eng is nc.vector:
                                    eng.tensor_copy(out_sb[:, o + c0 : o + c0 + 512], ps_t[:])
                                else:
                                    eng.copy(out_sb[:, o + c0 : o + c0 + 512], ps_t[:])
                        else:
                            ps = mp.tile([128, 1536], f32)
                            nc.tensor.matmul(ps[:, 0:512], uw0, g4_sb[:, gb : gb + 512])
                            nc.tensor.matmul(
                                ps[:, 512:768], uw0, g4_sb[:, gb + 512 : gb + 768]
                            )
                            nc.tensor.matmul(
                                ps[:, 768:1024], uw1, g4_sb[:, gb : gb + 256]
                            )
                            nc.tensor.matmul(
                                ps[:, 1024:1536], uw1, g4_sb[:, gb + 256 : gb + 768]
                            )
                            nc.vector.tensor_add(
                                out_sb[:, o : o + 1536], ps[:], pcst_sb[:]
                            )
                    nc.sync.dma_start(
                        out_d[grp[0] : grp[0] + nq].rearrange("q (p w) e -> p q w e", w=2),
                        out_sb[:].rearrange("p (q w e) -> p q w e", q=nq, w=2),
                    )
    nc.compile()
    return nc


def _prep_inputs(coords, mask, pos, w1, b1, w2, b2):
    nan0 = np.isnan(coords[..., 0])
    c = np.nan_to_num(coords)
    vis = np.where(nan0, np.float32(0.0), mask).astype(np.float32)

    p_all = np.ascontiguousarray(vis.transpose(0, 2, 1)).reshape(BT, M)
    c_bt = np.ascontiguousarray(c.transpose(0, 2, 1, 3)).reshape(BT, M, 2)
    q_all = (p_all[:, :, None] * c_bt).reshape(BT, 2 * M).astype(np.float32)

    W2t = w2[:D_MOT]
    W2b = w2[D_MOT:]
    cvec = (b1 @ W2t + b2).astype(np.float32)
    pcst = (pos @ W2b).astype(np.float32)
    variant = "fold" if not np.any(pcst) else "add"

    v2 = (w1 @ W2t).astype(np.float32)
    vw_dev = np.ascontiguousarray(
        v2.astype(BF16).reshape(4, 128, D_OUT).transpose(1, 0, 2)
    ).reshape(128, 4 * D_OUT)

    u0 = q_all[:, 0::2]
    u1 = q_all[:, 1::2]
    ones = np.ones_like(p_all)
    U4 = np.stack([u0, u1, -p_all, ones], axis=0)         # (4, BT, M)
    U4 = U4.reshape(4, BT, 128, 2).transpose(0, 1, 3, 2).astype(BF16)

    cv = np.broadcast_to(cvec.astype(BF16), (R, E)).reshape(1, R * E).copy()
    pcst_dev = None
    if variant == "add":
        pcst_dev = np.ascontiguousarray(pcst.reshape(128, 2, D_OUT)).reshape(128, 1536)

    qb = q_all.astype(BF16)

    in_maps = []
    for i in range(N_CORES):
        rows = slice(i * R, (i + 1) * R)
        pc_t = p_all[rows].T
        la = np.zeros((512, 96), np.float32)
        la[0::2, 0:32] = pc_t
        la[1::2, 32:64] = pc_t
        la = la.astype(BF16)
        la[:, 64:96] = qb[rows].T
        la_i = np.ascontiguousarray(
            la.reshape(4, 128, 96).transpose(1, 0, 2)
        ).reshape(128, 384)
        m = {
            "la": la_i,
            "vw": vw_dev,
            "ut4": np.ascontiguousarray(U4[:, rows]).reshape(4, R * 256),
            "cvd": cv,
        }
        if variant == "add":
            m["pcst"] = pcst_dev
        in_maps.append(m)
    return in_maps, variant


def _run(inputs, trace=False, trace_kwargs=None):
    from concourse.bass_utils import run_bass_kernel_spmd

    coords = np.asarray(inputs["point_trajs_gt_coord"], dtype=np.float32)
    mask = np.asarray(inputs["point_trajs_visibility_mask"], dtype=np.float32)
    pos = np.asarray(inputs["pos_embed"], dtype=np.float32)
    w1 = np.asarray(inputs["fc1_w"], dtype=np.float32)
    b1 = np.asarray(inputs["fc1_b"], dtype=np.float32)
    w2 = np.asarray(inputs["fc_out_w"], dtype=np.float32)
    b2 = np.asarray(inputs["fc_out_b"], dtype=np.float32)

    in_maps, variant = _prep_inputs(coords, mask, pos, w1, b1, w2, b2)
    if variant not in _CACHED_NC:
        _CACHED_NC[variant] = _build_nc(variant)
    nc = _CACHED_NC[variant]

    res = run_bass_kernel_spmd(
        nc, in_maps, list(range(N_CORES)), trace=trace, **(trace_kwargs or {})
    )
    shards = [np.asarray(res.results[i]["out"]) for i in range(N_CORES)]
    full = np.concatenate(shards, axis=0).astype(np.float32).reshape(B, T, M, D_OUT)
    return full, res


def kernel(**inputs):
    out, _ = _run(inputs, trace=False)
    return out


# revision 10
# speedup vs baseline: 1.3952x; 1.0118x over previous
"""Trainium2 Bass kernel for nn_CrossmotionModule (gnn_message_passing).

Reference computation (B=4, M=256, T=64, Dm=512, E=768):
    rel[b,m,t,n,k] = (c[b,m,t,k] - c[b,n,t,k]) * vis[b,m,t] * vis[b,n,t]
    fea[b,t,m,(n,k)] = rel                  # (B,T,M,512)
    h   = fea @ W1 + b1                     # (B,T,M,512)
    out = [h, pos] @ W2 + b2                # (B,T,M,768)

Algebraic collapse: with p = vis (B,T,M), u0 = p*c0, u1 = p*c1, the output is
a rank-3 outer product per (b,t) plus a constant:
    out[bt,m,e] = u0[m]*G0[e] + u1[m]*G1[e] - p[m]*G2[e] + const[m,e]
where, with the host-folded fused weight V2 = W1 @ W2[:512] (512, 768):
    G_j = [P0 | P1 | Q]_j^T V2  (bf16 inputs, fp32 PSUM accumulate)
    const = cvec + pos @ W2[512:],  cvec = b1 @ W2[:512] + b2

Everything runs in single bf16 (fp32 PSUM accumulate); the m-independent
const (cvec) is folded into the per-row matmul as a 4th contraction row
(ones x cvec). Output is written bf16 and widened to fp32 on host
(rel_l2 ~ 3.3e-3, gate 2e-2). The per-row PSUM is drained in three 512-col
bank-aligned chunks by three different engines (DVE / Act / GpSimd) so the
drain never caps the tensor engine. A few warmup matmuls on scratch data
keep the PE p-state up through the input-DMA wait.

When pos @ W2[512:] is nonzero a fallback variant adds the (m,e)-dependent
const on the vector engine instead (correct for any input, slower).

Sharding: data-parallel over bt = (b,t); 256 rows / 8 cores = 32 per core.
Weights replicated; no cross-device communication.
"""

import ml_dtypes
import numpy as np

B, M, T = 4, 256, 64
D_MOT, D_ABS, D_OUT = 512, 512, 768
N_CORES = 8
BT = B * T            # 256
R = BT // N_CORES     # 32 bt rows per core
E = D_OUT
RT = 2

BF16 = ml_dtypes.bfloat16

_CACHED_NC = {}


def _build_nc(variant):
    """variant: 'fold' — const is rank-1, folded into the matmul.
                'add'  — general const; epilogue adds pcst on DVE."""
    import concourse.bacc as bacc
    import concourse.bass as bass
    import concourse.mybir as mybir
    import concourse.tile as tile

    f32 = mybir.dt.float32
    bf16 = mybir.dt.bfloat16
    PSUM = bass.MemorySpace.PSUM

    nc = bacc.Bacc("TRN2", target_bir_lowering=False, debug=False)

    la_d = nc.dram_tensor("la", [128, 4 * 96], bf16, kind="ExternalInput")
    vw_d = nc.dram_tensor("vw", [128, 4 * E], bf16, kind="ExternalInput")
    ut_d = nc.dram_tensor("ut4", [4, R * 256], bf16, kind="ExternalInput")
    cv_d = nc.dram_tensor("cvd", [1, R * E], bf16, kind="ExternalInput")
    if variant == "add":
        pcst_d = nc.dram_tensor("pcst", [128, 1536], f32, kind="ExternalInput")
    out_d = nc.dram_tensor("out", [R, M, E], bf16, kind="ExternalOutput")

    with tile.TileContext(nc) as tc:
        with tc.tile_pool(name="persist", bufs=1) as pers:
            ut_sb = pers.tile([4, R * 256], bf16)
            g4_sb = pers.tile([4, R * E], bf16)
            if variant == "add":
                pcst_sb = pers.tile([128, 1536], f32)

            # ---- prologue: G[(j,r), e] = L^T V2 ----
            with (
                tc.tile_pool(name="pro", bufs=1) as pro,
                tc.tile_pool(name="prop", bufs=1, space=PSUM) as prop,
            ):
                la_sb = pro.tile([128, 4 * 96], bf16)
                vw_sb = pro.tile([128, 4 * E], bf16)
                gtmp = pro.tile([96, E], bf16)

                # Two fat half-DMAs for vw (6 KB/partition descriptors beat
                # four skinny chunk reads), one per HWDGE queue; small inputs
                # ride the Act queue up front.
                nc.sync.dma_start(vw_sb[:, 0 : 2 * E], vw_d[:, 0 : 2 * E])
                nc.scalar.dma_start(vw_sb[:, 2 * E : 4 * E], vw_d[:, 2 * E : 4 * E])
                nc.sync.dma_start(la_sb[:], la_d[:])
                nc.scalar.dma_start(ut_sb[:], ut_d[:])
                nc.scalar.dma_start(g4_sb[3:4, :], cv_d[:])
                if variant == "add":
                    nc.scalar.dma_start(pcst_sb[:], pcst_d[:])

                gps = prop.tile([96, E], f32)
                for kk in range(4):
                    for lo, hi in ((0, 512), (512, E)):
                        nc.tensor.matmul(
                            gps[:, lo:hi],
                            la_sb[:, kk * 96 : (kk + 1) * 96],
                            vw_sb[:, kk * E + lo : kk * E + hi],
                            start=(kk == 0),
                            stop=(kk == 3),
                        )
                nc.vector.tensor_copy(gtmp[:, 0:384], gps[:, 0:384])
                nc.scalar.copy(gtmp[:, 384:E], gps[:, 384:E])

                # Reshuffle rows (j*R + r, e) -> (j, r*E + e), SBUF->SBUF,
                # r-chunked so early rows unblock first.
                CK = (R // 2) * E
                for ck in range(2):
                    for j in range(3):
                        eng = nc.sync if (ck * 3 + j) % 2 == 0 else nc.scalar
                        eng.dma_start(
                            g4_sb[j : j + 1, ck * CK : (ck + 1) * CK],
                            gtmp[j * R + ck * 16 : j * R + (ck + 1) * 16, :],
                        )

            # ---- main loop ----
            groups = [list(range(g, g + RT)) for g in range(0, R - 4, RT)] + [
                [r] for r in range(R - 4, R)
            ]
            with (
                tc.tile_pool(name="mp", bufs=2, space=PSUM) as mp,
                tc.tile_pool(name="op", bufs=4) as op,
            ):
                for grp in groups:
                    nq = len(grp)
                    out_sb = op.tile([128, nq * 1536], bf16, tag="out_sb")
                    for q, r in enumerate(grp):
                        o = q * 1536
                        uw0 = ut_sb[:, r * 256 : r * 256 + 128]
                        uw1 = ut_sb[:, r * 256 + 128 : r * 256 + 256]
                        gb = r * E
                        if variant == "fold":
                            psa = mp.tile([128, 512], f32)
                            psb = mp.tile([128, 512], f32)
                            psc = mp.tile([128, 512], f32)
                            nc.tensor.matmul(psa[:], uw0, g4_sb[:, gb : gb + 512])
                            nc.tensor.matmul(
                                psb[:, 0:256], uw0, g4_sb[:, gb + 512 : gb + 768]
                            )
                            nc.tensor.matmul(
                                psb[:, 256:512], uw1, g4_sb[:, gb : gb + 256]
                            )
                            nc.tensor.matmul(psc[:], uw1, g4_sb[:, gb + 256 : gb + 768])
                            e1, e2 = (
                                (nc.vector, nc.scalar)
                                if r % 2 == 0
                                else (nc.scalar, nc.vector)
                            )
                            for eng, ps_t, c0 in ((e1, psa, 0), (e2, psb, 512), (e1, psc, 1024)):
                                if eng is nc.vector:
                                    eng.tensor_copy(out_sb[:, o + c0 : o + c0 + 512], ps_t[:])
                                else:
                                    eng.copy(out_sb[:, o + c0 : o + c0 + 512], ps_t[:])
                        else:
                            ps = mp.tile([128, 1536], f32)
                            nc.tensor.matmul(ps[:, 0:512], uw0, g4_sb[:, gb : gb + 512])
                            nc.tensor.matmul(
                                ps[:, 512:768], uw0, g4_sb[:, gb + 512 : gb + 768]
                            )
                            nc.tensor.matmul(
                                ps[:, 768:1024], uw1, g4_sb[:, gb : gb + 256]
                            )
                            nc.tensor.matmul(
                                ps[:, 1024:1536], uw1, g4_sb[:, gb + 256 : gb + 768]
                            )
                            nc.vector.tensor_add(
                                out_sb[:, o : o + 1536], ps[:], pcst_sb[:]
                            )
                    nc.sync.dma_start(
                        out_d[grp[0] : grp[0] + nq].rearrange("q (p w) e -> p q w e", w=2),
                        out_sb[:].rearrange("p (q w e) -> p q w e", q=nq, w=2),
                    )
    nc.compile()
    return nc


def _prep_inputs(coords, mask, pos, w1, b1, w2, b2):
    nan0 = np.isnan(coords[..., 0])
    c = np.nan_to_num(coords)
    vis = np.where(nan0, np.float32(0.0), mask).astype(np.float32)

    p_all = np.ascontiguousarray(vis.transpose(0, 2, 1)).reshape(BT, M)
    c_bt = np.ascontiguousarray(c.transpose(0, 2, 1, 3)).reshape(BT, M, 2)
    q_all = (p_all[:, :, None] * c_bt).reshape(BT, 2 * M).astype(np.float32)

    W2t = w2[:D_MOT]
    W2b = w2[D_MOT:]
    cvec = (b1 @ W2t + b2).astype(np.float32)
    pcst = (pos @ W2b).astype(np.float32)
    variant = "fold" if not np.any(pcst) else "add"

    v2 = (w1 @ W2t).astype(np.float32)
    vw_dev = np.ascontiguousarray(
        v2.astype(BF16).reshape(4, 128, D_OUT).transpose(1, 0, 2)
    ).reshape(128, 4 * D_OUT)

    u0 = q_all[:, 0::2]
    u1 = q_all[:, 1::2]
    ones = np.ones_like(p_all)
    U4 = np.stack([u0, u1, -p_all, ones], axis=0)         # (4, BT, M)
    U4 = U4.reshape(4, BT, 128, 2).transpose(0, 1, 3, 2).astype(BF16)

    cv = np.broadcast_to(cvec.astype(BF16), (R, E)).reshape(1, R * E).copy()
    pcst_dev = None
    if variant == "add":
        pcst_dev = np.ascontiguousarray(pcst.reshape(128, 2, D_OUT)).reshape(128, 1536)

    qb = q_all.astype(BF16)

    in_maps = []
    for i in range(N_CORES):
        rows = slice(i * R, (i + 1) * R)
        pc_t = p_all[rows].T
        la = np.zeros((512, 96), np.float32)
        la[0::2, 0:32] = pc_t
        la[1::2, 32:64] = pc_t
        la = la.astype(BF16)
        la[:, 64:96] = qb[rows].T
        la_i = np.ascontiguousarray(
            la.reshape(4, 128, 96).transpose(1, 0, 2)
        ).reshape(128, 384)
        m = {
            "la": la_i,
            "vw": vw_dev,
            "ut4": np.ascontiguousarray(U4[:, rows]).reshape(4, R * 256),
            "cvd": cv,
        }
        if variant == "add":
            m["pcst"] = pcst_dev
        in_maps.append(m)
    return in_maps, variant


def _run(inputs, trace=False, trace_kwargs=None):
    from concourse.bass_utils import run_bass_kernel_spmd

    coords = np.asarray(inputs["point_trajs_gt_coord"], dtype=np.float32)
    mask = np.asarray(inputs["point_trajs_visibility_mask"], dtype=np.float32)
    pos = np.asarray(inputs["pos_embed"], dtype=np.float32)
    w1 = np.asarray(inputs["fc1_w"], dtype=np.float32)
    b1 = np.asarray(inputs["fc1_b"], dtype=np.float32)
    w2 = np.asarray(inputs["fc_out_w"], dtype=np.float32)
    b2 = np.asarray(inputs["fc_out_b"], dtype=np.float32)

    in_maps, variant = _prep_inputs(coords, mask, pos, w1, b1, w2, b2)
    if variant not in _CACHED_NC:
        _CACHED_NC[variant] = _build_nc(variant)
    nc = _CACHED_NC[variant]

    res = run_bass_kernel_spmd(
        nc, in_maps, list(range(N_CORES)), trace=trace, **(trace_kwargs or {})
    )
    shards = [np.asarray(res.results[i]["out"]) for i in range(N_CORES)]
    full = np.concatenate(shards, axis=0).astype(np.float32).reshape(B, T, M, D_OUT)
    return full, res


def kernel(**inputs):
    out, _ = _run(inputs, trace=False)
    return out
